# revision 21
# baseline (speedup 1.0000x reference)
# GemmaAttention on 8 Trainium2 NeuronCores — batch-DP x 4-head-TP.
#
# Sharding: core c -> batch b = c//4, head group g = c%4. Each core owns
# Q heads 4g..4g+3 (GQA group g exactly), KV head g, and tokens of batch
# b only. No redundant KV compute (38.7 GFLOP/core, the exact 1/8 of the
# model). Wq/Wk/Wv sliced column-wise, Wo row-wise; host sums 4 partials
# per batch. K and V stay SBUF-resident (no DRAM spill).
#
# Device kernel (per core, bf16 matmuls, f32 PSUM):
#   warmup) dummy matmuls overlap the initial weight/x DMA and keep the
#      HAM clock gate at 2.4 GHz from the first real matmul on.
#   A) QKV projection + RoPE: stream xT token tiles, accumulate q/k/v in
#      PSUM over the 2048 contraction, RoPE on drain (DVE). q/k/v all
#      stay in SBUF.
#   B) attention, software-pipelined ACROSS (head, query-tile) tiles:
#      the next tile's first two score groups are emitted before the
#      previous tile's trailing yP/finalize matmuls so the in-order PE
#      queue never waits on ScalarE exp at a tile boundary. Softmax
#      denominator: bf16 accumulate chain on DVE + rank-1 ones matmuls +
#      reciprocal_approx_fast; normalization folded into the yP
#      PSUM->SBUF drain (DVE).
#   C) output projection, emitted inside the B scope: reuses the score
#      PSUM banks, one row-wide [128, 2048] bf16 staging tile and a
#      single 512KB DMA per 128-token block; the last B finalize is
#      sandwiched between the first C tiles so nothing stalls.
import numpy as np
import ml_dtypes
from contextlib import ExitStack

import concourse.bass as bass
import concourse.mybir as mybir
import concourse.tile as tile
from concourse import bacc
from concourse.bass_utils import run_bass_kernel_spmd

P = 128
F32 = mybir.dt.float32
F32R = mybir.dt.float32r
BF16 = mybir.dt.bfloat16
EXP = mybir.ActivationFunctionType.Exp

B, T, C = 2, 2048, 2048
H, KV, D = 16, 4, 256
THETA = 10000.0
NH = H // KV            # 4 q heads per core
NCC = C // P            # 16 contraction chunks
TT_A = 512              # phase-A token tile
NT_A = T // TT_A        # 4
TQ = 512                # phase-B query tile
NTQ = T // TQ           # 4
NKC = T // P            # 16 key chunks
NT_B = NH * NTQ         # 16 attention tiles
SCALE = 1.0 / 16.0      # 1/sqrt(D)
FQ = 2 * NH             # 8 q feature chunks per core (4 heads x 256)
WCOLS = NH * D + 2 * D  # 1536 wqkv columns per core
NVC = T // P            # 16 v token chunks


def build():
    nc = bacc.Bacc("TRN2", target_bir_lowering=False, debug=False)
    xT = nc.dram_tensor("xT", [C, T], BF16, kind="ExternalInput").ap()
    cosT = nc.dram_tensor("cosT", [P, T], F32, kind="ExternalInput").ap()
    sinT = nc.dram_tensor("sinT", [P, T], F32, kind="ExternalInput").ap()
    wqkv = nc.dram_tensor("wqkv", [C, WCOLS], BF16, kind="ExternalInput").ap()
    wo = nc.dram_tensor("wo", [NH * D, C], BF16, kind="ExternalInput").ap()
    out = nc.dram_tensor("out", [T, C], BF16, kind="ExternalOutput").ap()

    xT3 = xT.rearrange("(o p) t -> p o t", p=P)        # [128, 16, 2048]
    wqkv3 = wqkv.rearrange("(o p) f -> p o f", p=P)    # [128, 16, 1536]
    wo3 = wo.rearrange("(o p) f -> p o f", p=P)        # [128, 8, 2048]

    with tile.TileContext(nc) as tc, ExitStack() as octx:
        const = octx.enter_context(tc.tile_pool(name="const", bufs=1))
        ones_f = const.tile([P, 1], F32)
        nc.vector.memset(ones_f[:], 1.0)
        ones_col = const.tile([P, 1], BF16)
        nc.vector.tensor_copy(ones_col[:], ones_f[:])
        ones_row_f = const.tile([1, P], F32)
        nc.vector.memset(ones_row_f[:], 1.0)
        ones_row = const.tile([1, P], BF16)
        nc.vector.tensor_copy(ones_row[:], ones_row_f[:])
        wtile_f = const.tile([P, D], F32)
        nc.vector.memset(wtile_f[:], 0.25)
        wtile = const.tile([P, D], BF16)
        nc.vector.tensor_copy(wtile[:], wtile_f[:])

        # resident across phases
        qres = octx.enter_context(tc.tile_pool(name="qres", bufs=1))
        qT = qres.tile([P, FQ, T], BF16)           # 32KB/part
        kT = qres.tile([P, 2, T], BF16)            # 8KB/part
        v_sb = qres.tile([P, NVC, D], BF16)        # 8KB/part

        # ---------------- Phase A: QKV projection + RoPE ----------------
        with ExitStack() as actx:
            wq_pool = actx.enter_context(tc.tile_pool(name="wq", bufs=1))
            wqkv_sb = wq_pool.tile([P, NCC, WCOLS], BF16)   # 48KB/part
            for cc in range(4):
                nc.sync.dma_start(wqkv_sb[:, cc, :], wqkv3[:, cc, :])

            xt_pool = actx.enter_context(tc.tile_pool(name="xt", bufs=2))
            cs_pool = actx.enter_context(tc.tile_pool(name="cs", bufs=2))
            tmp_pool = actx.enter_context(tc.tile_pool(name="tmp", bufs=2))
            aps = actx.enter_context(
                tc.tile_pool(name="apsum", bufs=4, space="PSUM"))
            vps_pool = actx.enter_context(
                tc.tile_pool(name="vpsum", bufs=2, space="PSUM"))
            wps = actx.enter_context(
                tc.tile_pool(name="wpsum", bufs=1, space="PSUM"))

            # warm the PE / HAM while the initial DMAs land (~13us)
            warm_ps = wps.tile([P, TQ], F32)
            for _ in range(112):
                nc.tensor.matmul(warm_ps[:, :D], wtile[:, :P], wtile[:],
                                 start=True, stop=True)

            for tt in range(NT_A):
                t0 = tt * TT_A
                xt = xt_pool.tile([P, NCC, TT_A], BF16, tag="xt")
                for g4 in range(4):
                    nc.sync.dma_start(
                        xt[:, 4 * g4:4 * g4 + 4, :],
                        xT3[:, 4 * g4:4 * g4 + 4, t0:t0 + TT_A])
                if tt == 0:
                    for cc in range(4, NCC):
                        nc.sync.dma_start(wqkv_sb[:, cc, :], wqkv3[:, cc, :])
                cos_t = cs_pool.tile([P, TT_A], F32, tag="cos")
                nc.sync.dma_start(cos_t[:], cosT[:, t0:t0 + TT_A])
                sin_t = cs_pool.tile([P, TT_A], F32, tag="sin")
                nc.sync.dma_start(sin_t[:], sinT[:, t0:t0 + TT_A])

                def rope(lo_ps, hi_ps, lo_out, hi_out):
                    # lo' = lo*cos - hi*sin ; hi' = hi*cos + lo*sin
                    ta = tmp_pool.tile([P, TT_A], F32, tag="ra")
                    tb = tmp_pool.tile([P, TT_A], F32, tag="rb")
                    nc.vector.tensor_mul(ta[:], lo_ps, cos_t[:])
                    nc.vector.tensor_mul(tb[:], hi_ps, sin_t[:])
                    nc.vector.tensor_sub(lo_out, ta[:], tb[:])
                    tc2 = tmp_pool.tile([P, TT_A], F32, tag="ra")
                    td = tmp_pool.tile([P, TT_A], F32, tag="rb")
                    nc.vector.tensor_mul(tc2[:], hi_ps, cos_t[:])
                    nc.vector.tensor_mul(td[:], lo_ps, sin_t[:])
                    nc.vector.tensor_add(hi_out, tc2[:], td[:])

                # q: 8 feature chunks (4 heads x 2 halves), rope per pair
                for h in range(NH):
                    qp = [None, None]
                    for dc in range(2):
                        j = 2 * h + dc
                        ps = aps.tile([P, TT_A], F32, tag="ps")
                        for cc in range(NCC):
                            nc.tensor.matmul(
                                ps[:], wqkv_sb[:, cc, j * P:(j + 1) * P],
                                xt[:, cc, :],
                                start=(cc == 0), stop=(cc == NCC - 1))
                        qp[dc] = ps
                    rope(qp[0][:], qp[1][:],
                         qT[:, 2 * h, t0:t0 + TT_A],
                         qT[:, 2 * h + 1, t0:t0 + TT_A])

                # k: 2 feature chunks, one rope pair, straight into kT
                kp = [None, None]
                for dc in range(2):
                    ps = aps.tile([P, TT_A], F32, tag="ps")
                    for cc in range(NCC):
                        nc.tensor.matmul(
                            ps[:],
                            wqkv_sb[:, cc, NH * D + dc * P:NH * D + (dc + 1) * P],
                            xt[:, cc, :],
                            start=(cc == 0), stop=(cc == NCC - 1))
                    kp[dc] = ps
                rope(kp[0][:], kp[1][:],
                     kT[:, 0, t0:t0 + TT_A], kT[:, 1, t0:t0 + TT_A])

                # v: 4 token sub-chunks, no rope, straight into v_sb
                for s in range(4):
                    ps = vps_pool.tile([P, D], F32, tag="vps")
                    for cc in range(NCC):
                        nc.tensor.matmul(
                            ps[:], xt[:, cc, s * P:(s + 1) * P],
                            wqkv_sb[:, cc, NH * D + 2 * P:NH * D + 2 * P + D],
                            start=(cc == 0), stop=(cc == NCC - 1))
                    nc.scalar.copy(v_sb[:, 4 * tt + s, :], ps[:])

        # yT and wo_sb live in the address space wqkv_sb vacated
        with ExitStack() as bctx:
            yres = bctx.enter_context(tc.tile_pool(name="yres", bufs=1))
            yT = yres.tile([P, FQ, T], BF16)           # 32KB/part
            wo_sb = yres.tile([P, FQ, C], BF16)        # 32KB/part
            for dc in range(FQ):
                nc.sync.dma_start(wo_sb[:, dc, :], wo3[:, dc, :])

            # ------------- Phase B+C: attention + output projection -----
            pb_pool = bctx.enter_context(tc.tile_pool(name="pb", bufs=3))
            red_pool = bctx.enter_context(tc.tile_pool(name="red", bufs=2))
            acc_pool = bctx.enter_context(tc.tile_pool(name="acc", bufs=2))
            ost_pool = bctx.enter_context(tc.tile_pool(name="ost", bufs=2))
            sps_pool = bctx.enter_context(
                tc.tile_pool(name="spsum", bufs=2, space="PSUM"))
            yps_pool = bctx.enter_context(
                tc.tile_pool(name="ypsum", bufs=3, space="PSUM"))
            rps = bctx.enter_context(
                tc.tile_pool(name="rpsum", bufs=1, space="PSUM"))

            state = [None] * NT_B

            def tile_hq(t):
                return t // NTQ, (t % NTQ) * TQ

            def open_tile(t):
                state[t] = {
                    "pbs": [None] * NKC,
                    "acc": acc_pool.tile([P, TQ], BF16, tag="acc", name="acc"),
                    "accr": red_pool.tile([P, TQ], BF16, tag="accr",
                                          name="accr"),
                    "yp": [yps_pool.tile([P, TQ], F32, tag="yp",
                                         name=f"yp{i}") for i in range(2)],
                    "s_row": None,
                }

            def emit_sps_pair(t, j):
                # two key chunks' score matmuls into one 2-bank PSUM tile,
                # drained by a single wide exp (halves ScalarE overhead)
                h, tq0 = tile_hq(t)
                st = state[t]
                sps = sps_pool.tile([P, 2, TQ], F32, tag="sps")
                pb = pb_pool.tile([P, 2, TQ], BF16, tag="pb")
                for u in range(2):
                    kc = 2 * j + u
                    for dc in range(2):
                        nc.tensor.matmul(
                            sps[:, u, :], kT[:, dc, kc * P:(kc + 1) * P],
                            qT[:, 2 * h + dc, tq0:tq0 + TQ],
                            start=(dc == 0), stop=(dc == 1))
                    st["pbs"][kc] = pb[:, u, :]
                nc.scalar.activation(pb[:], sps[:], EXP, scale=SCALE)

            def emit_yp(t, kc):
                st = state[t]
                for dc in range(2):
                    nc.tensor.matmul(
                        st["yp"][dc][:], v_sb[:, kc, dc * P:(dc + 1) * P],
                        st["pbs"][kc],
                        start=(kc == 0), stop=(kc == NKC - 1))

            def emit_add(t, kc):
                st = state[t]
                if kc == 1:
                    nc.vector.tensor_add(
                        st["acc"][:], st["pbs"][0], st["pbs"][1])
                elif kc == NKC - 1:
                    nc.vector.tensor_add(
                        st["accr"][:], st["acc"][:], st["pbs"][kc])
                else:
                    nc.vector.tensor_add(
                        st["acc"][:], st["acc"][:], st["pbs"][kc])

            def fin1(t):
                st = state[t]
                emit_add(t, NKC - 1)
                sum_ps = rps.tile([1, TQ], F32, tag="r", name="sum_ps")
                nc.tensor.matmul(sum_ps[:], ones_col[:], st["accr"][:],
                                 start=True, stop=True)
                s_row = red_pool.tile([1, TQ], BF16, tag="srow")
                nc.vector.tensor_copy(s_row[:], sum_ps[:])
                st["s_row"] = s_row

            def fin2(t):
                st = state[t]
                h, tq0 = tile_hq(t)
                bc_ps = rps.tile([P, TQ], F32, tag="r", name="bc_ps")
                nc.tensor.matmul(bc_ps[:], ones_row[:], st["s_row"][:],
                                 start=True, stop=True)
                rcp = red_pool.tile([P, TQ], F32, tag="rcp")
                nc.vector.reciprocal_approx_fast(rcp[:], bc_ps[:])
                for dc in range(2):
                    nc.vector.tensor_mul(
                        yT[:, 2 * h + dc, tq0:tq0 + TQ],
                        st["yp"][dc][:], rcp[:])
                state[t] = None

            for t in range(NT_B):
                open_tile(t)
                emit_sps_pair(t, 0)
                if t > 0:
                    emit_yp(t - 1, NKC - 3)
                    emit_yp(t - 1, NKC - 2)
                    emit_yp(t - 1, NKC - 1)
                    fin1(t - 1)
                    fin2(t - 1)
                for kc in range(2, NKC):
                    if kc % 2 == 0:
                        emit_sps_pair(t, kc // 2)
                    emit_add(t, kc - 1)
                    if kc >= 3:
                        emit_yp(t, kc - 3)

            # phase C tiles (reuse the sps PSUM banks; the last attention
            # finalize is sandwiched between the first C tiles)
            def emit_c(tch):
                ot = ost_pool.tile([P, C], BF16, tag="ot")
                for cop in range(2):
                    ps = sps_pool.tile([P, 2, TQ], F32, tag="sps", name="cps")
                    for u in range(2):
                        co = 2 * cop + u
                        for dc in range(FQ):
                            nc.tensor.matmul(
                                ps[:, u, :], yT[:, dc, tch * P:(tch + 1) * P],
                                wo_sb[:, dc, co * TQ:(co + 1) * TQ],
                                start=(dc == 0), stop=(dc == FQ - 1))
                        nc.scalar.copy(ot[:, co * TQ:(co + 1) * TQ],
                                       ps[:, u, :])
                nc.sync.dma_start(out[tch * P:(tch + 1) * P, :], ot[:])

            tl = NT_B - 1
            emit_yp(tl, NKC - 3)
            emit_yp(tl, NKC - 2)
            emit_yp(tl, NKC - 1)
            fin1(tl)
            emit_c(0)
            emit_c(1)
            fin2(tl)
            for tch in range(2, T // P):
                emit_c(tch)

    nc.compile()
    return nc


_NC = None
_TRACE = False      # set by test harness to capture an NTFF profile
_LAST_RES = None


def _get_nc():
    global _NC
    if _NC is None:
        _NC = build()
    return _NC


def kernel(x, position_ids, Wq, Wk, Wv, Wo):
    x = np.ascontiguousarray(np.asarray(x, dtype=np.float32))
    pos = np.asarray(position_ids)
    Wq = np.asarray(Wq, dtype=np.float32)
    Wk = np.asarray(Wk, dtype=np.float32)
    Wv = np.asarray(Wv, dtype=np.float32)
    Wo = np.asarray(Wo, dtype=np.float32)

    inv = 1.0 / (THETA ** (np.arange(0, D, 2, dtype=np.float64) / D))  # [128]

    in_maps = []
    for c in range(8):
        b, g = divmod(c, 4)
        xTb = np.ascontiguousarray(x[b].T).astype(ml_dtypes.bfloat16)  # [C,T]
        ang = inv[:, None] * pos[b].astype(np.float64)[None, :]        # [128,T]
        cosT = np.cos(ang).astype(np.float32)
        sinT = np.sin(ang).astype(np.float32)
        wqkv_np = np.ascontiguousarray(np.concatenate(
            [Wq[:, g * 1024:(g + 1) * 1024],
             Wk[:, g * 256:(g + 1) * 256],
             Wv[:, g * 256:(g + 1) * 256]], axis=1)).astype(ml_dtypes.bfloat16)
        wo_np = np.ascontiguousarray(
            Wo[g * 1024:(g + 1) * 1024, :]).astype(ml_dtypes.bfloat16)
        in_maps.append({"xT": xTb, "cosT": cosT, "sinT": sinT,
                        "wqkv": wqkv_np, "wo": wo_np})

    nc = _get_nc()
    res = run_bass_kernel_spmd(nc, in_maps, core_ids=list(range(8)),
                               trace=_TRACE)
    global _LAST_RES
    _LAST_RES = res
    outs = [r["out"].astype(np.float32) for r in res.results]
    return np.stack([outs[0] + outs[1] + outs[2] + outs[3],
                     outs[4] + outs[5] + outs[6] + outs[7]])


# revision 25
# speedup vs baseline: 1.0119x; 1.0119x over previous
# GemmaAttention on 8 Trainium2 NeuronCores — batch-DP x 4-head-TP.
#
# Sharding: core c -> batch b = c//4, head group g = c%4. Each core owns
# Q heads 4g..4g+3 (GQA group g exactly), KV head g, and tokens of batch
# b only. No redundant KV compute (38.7 GFLOP/core, the exact 1/8 of the
# model). Wq/Wk/Wv sliced column-wise, Wo row-wise; host sums 4 partials
# per batch. K and V stay SBUF-resident (no DRAM spill).
#
# Device kernel (per core, bf16 matmuls, f32 PSUM):
#   warmup) dummy matmuls overlap the initial weight/x DMA and keep the
#      HAM clock gate at 2.4 GHz from the first real matmul on.
#   A) QKV projection + RoPE: stream xT token tiles, accumulate q/k/v in
#      PSUM over the 2048 contraction, RoPE on drain (DVE). q/k/v all
#      stay in SBUF.
#   B) attention, software-pipelined ACROSS (head, query-tile) tiles:
#      the next tile's first two score groups are emitted before the
#      previous tile's trailing yP/finalize matmuls so the in-order PE
#      queue never waits on ScalarE exp at a tile boundary. Softmax
#      denominator: bf16 accumulate chain on DVE + rank-1 ones matmuls +
#      reciprocal_approx_fast; normalization folded into the yP
#      PSUM->SBUF drain (DVE).
#   C) output projection, emitted inside the B scope: reuses the score
#      PSUM banks, one row-wide [128, 2048] bf16 staging tile and a
#      single 512KB DMA per 128-token block; the last B finalize is
#      sandwiched between the first C tiles so nothing stalls.
import numpy as np
import ml_dtypes
from contextlib import ExitStack

import concourse.bass as bass
import concourse.mybir as mybir
import concourse.tile as tile
from concourse import bacc
from concourse.bass_utils import run_bass_kernel_spmd

P = 128
F32 = mybir.dt.float32
F32R = mybir.dt.float32r
BF16 = mybir.dt.bfloat16
EXP = mybir.ActivationFunctionType.Exp

B, T, C = 2, 2048, 2048
H, KV, D = 16, 4, 256
THETA = 10000.0
NH = H // KV            # 4 q heads per core
NCC = C // P            # 16 contraction chunks
TT_A = 512              # phase-A token tile
NT_A = T // TT_A        # 4
TQ = 512                # phase-B query tile
NTQ = T // TQ           # 4
NKC = T // P            # 16 key chunks
NT_B = NH * NTQ         # 16 attention tiles
SCALE = 1.0 / 16.0      # 1/sqrt(D)
FQ = 2 * NH             # 8 q feature chunks per core (4 heads x 256)
WCOLS = NH * D + 2 * D  # 1536 wqkv columns per core
NVC = T // P            # 16 v token chunks


def build():
    nc = bacc.Bacc("TRN2", target_bir_lowering=False, debug=False)
    xT = nc.dram_tensor("xT", [C, T], BF16, kind="ExternalInput").ap()
    cosT = nc.dram_tensor("cosT", [P, T], F32, kind="ExternalInput").ap()
    sinT = nc.dram_tensor("sinT", [P, T], F32, kind="ExternalInput").ap()
    wqkv = nc.dram_tensor("wqkv", [C, WCOLS], BF16, kind="ExternalInput").ap()
    wo = nc.dram_tensor("wo", [NH * D, C], BF16, kind="ExternalInput").ap()
    out = nc.dram_tensor("out", [T, C], BF16, kind="ExternalOutput").ap()

    xT3 = xT.rearrange("(o p) t -> p o t", p=P)        # [128, 16, 2048]
    wqkv3 = wqkv.rearrange("(o p) f -> p o f", p=P)    # [128, 16, 1536]
    wo3 = wo.rearrange("(o p) f -> p o f", p=P)        # [128, 8, 2048]

    with tile.TileContext(nc) as tc, ExitStack() as octx:
        const = octx.enter_context(tc.tile_pool(name="const", bufs=1))
        ones_f = const.tile([P, P], F32)
        nc.vector.memset(ones_f[:], 1.0)
        ones_mat = const.tile([P, P], BF16)
        nc.vector.tensor_copy(ones_mat[:], ones_f[:])
        wtile_f = const.tile([P, D], F32)
        nc.vector.memset(wtile_f[:], 0.25)
        wtile = const.tile([P, D], BF16)
        nc.vector.tensor_copy(wtile[:], wtile_f[:])

        # resident across phases
        qres = octx.enter_context(tc.tile_pool(name="qres", bufs=1))
        qT = qres.tile([P, FQ, T], BF16)           # 32KB/part
        kT = qres.tile([P, 2, T], BF16)            # 8KB/part
        v_sb = qres.tile([P, NVC, D], BF16)        # 8KB/part

        # ---------------- Phase A: QKV projection + RoPE ----------------
        with ExitStack() as actx:
            wq_pool = actx.enter_context(tc.tile_pool(name="wq", bufs=1))
            wqkv_sb = wq_pool.tile([P, NCC, WCOLS], BF16)   # 48KB/part
            for cc in range(4):
                nc.sync.dma_start(wqkv_sb[:, cc, :], wqkv3[:, cc, :])

            xt_pool = actx.enter_context(tc.tile_pool(name="xt", bufs=2))
            cs_pool = actx.enter_context(tc.tile_pool(name="cs", bufs=2))
            tmp_pool = actx.enter_context(tc.tile_pool(name="tmp", bufs=2))
            aps = actx.enter_context(
                tc.tile_pool(name="apsum", bufs=4, space="PSUM"))
            vps_pool = actx.enter_context(
                tc.tile_pool(name="vpsum", bufs=2, space="PSUM"))
            wps = actx.enter_context(
                tc.tile_pool(name="wpsum", bufs=1, space="PSUM"))

            # warm the PE / HAM while the initial DMAs land (~13us)
            warm_ps = wps.tile([P, TQ], F32)
            for _ in range(112):
                nc.tensor.matmul(warm_ps[:, :D], wtile[:, :P], wtile[:],
                                 start=True, stop=True)

            for tt in range(NT_A):
                t0 = tt * TT_A
                xt = xt_pool.tile([P, NCC, TT_A], BF16, tag="xt")
                for g4 in range(4):
                    nc.sync.dma_start(
                        xt[:, 4 * g4:4 * g4 + 4, :],
                        xT3[:, 4 * g4:4 * g4 + 4, t0:t0 + TT_A])
                if tt == 0:
                    for cc in range(4, NCC):
                        nc.sync.dma_start(wqkv_sb[:, cc, :], wqkv3[:, cc, :])
                cos_t = cs_pool.tile([P, TT_A], F32, tag="cos")
                nc.sync.dma_start(cos_t[:], cosT[:, t0:t0 + TT_A])
                sin_t = cs_pool.tile([P, TT_A], F32, tag="sin")
                nc.sync.dma_start(sin_t[:], sinT[:, t0:t0 + TT_A])

                def rope(lo_ps, hi_ps, lo_out, hi_out):
                    # lo' = lo*cos - hi*sin ; hi' = hi*cos + lo*sin
                    ta = tmp_pool.tile([P, TT_A], F32, tag="ra")
                    tb = tmp_pool.tile([P, TT_A], F32, tag="rb")
                    nc.vector.tensor_mul(ta[:], lo_ps, cos_t[:])
                    nc.vector.tensor_mul(tb[:], hi_ps, sin_t[:])
                    nc.vector.tensor_sub(lo_out, ta[:], tb[:])
                    tc2 = tmp_pool.tile([P, TT_A], F32, tag="ra")
                    td = tmp_pool.tile([P, TT_A], F32, tag="rb")
                    nc.vector.tensor_mul(tc2[:], hi_ps, cos_t[:])
                    nc.vector.tensor_mul(td[:], lo_ps, sin_t[:])
                    nc.vector.tensor_add(hi_out, tc2[:], td[:])

                # q: 8 feature chunks (4 heads x 2 halves), rope per pair
                for h in range(NH):
                    qp = [None, None]
                    for dc in range(2):
                        j = 2 * h + dc
                        ps = aps.tile([P, TT_A], F32, tag="ps")
                        for cc in range(NCC):
                            nc.tensor.matmul(
                                ps[:], wqkv_sb[:, cc, j * P:(j + 1) * P],
                                xt[:, cc, :],
                                start=(cc == 0), stop=(cc == NCC - 1))
                        qp[dc] = ps
                    rope(qp[0][:], qp[1][:],
                         qT[:, 2 * h, t0:t0 + TT_A],
                         qT[:, 2 * h + 1, t0:t0 + TT_A])

                # k: 2 feature chunks, one rope pair, straight into kT
                kp = [None, None]
                for dc in range(2):
                    ps = aps.tile([P, TT_A], F32, tag="ps")
                    for cc in range(NCC):
                        nc.tensor.matmul(
                            ps[:],
                            wqkv_sb[:, cc, NH * D + dc * P:NH * D + (dc + 1) * P],
                            xt[:, cc, :],
                            start=(cc == 0), stop=(cc == NCC - 1))
                    kp[dc] = ps
                rope(kp[0][:], kp[1][:],
                     kT[:, 0, t0:t0 + TT_A], kT[:, 1, t0:t0 + TT_A])

                # v: 4 token sub-chunks, no rope, straight into v_sb
                for s in range(4):
                    ps = vps_pool.tile([P, D], F32, tag="vps")
                    for cc in range(NCC):
                        nc.tensor.matmul(
                            ps[:], xt[:, cc, s * P:(s + 1) * P],
                            wqkv_sb[:, cc, NH * D + 2 * P:NH * D + 2 * P + D],
                            start=(cc == 0), stop=(cc == NCC - 1))
                    nc.scalar.copy(v_sb[:, 4 * tt + s, :], ps[:])

        # yT and wo_sb live in the address space wqkv_sb vacated
        with ExitStack() as bctx:
            yres = bctx.enter_context(tc.tile_pool(name="yres", bufs=1))
            yT = yres.tile([P, FQ, T], BF16)           # 32KB/part
            wo_sb = yres.tile([P, FQ, C], BF16)        # 32KB/part
            for dc in range(FQ):
                nc.sync.dma_start(wo_sb[:, dc, :], wo3[:, dc, :])

            # ------------- Phase B+C: attention + output projection -----
            pb_pool = bctx.enter_context(tc.tile_pool(name="pb", bufs=3))
            red_pool = bctx.enter_context(tc.tile_pool(name="red", bufs=2))
            acc_pool = bctx.enter_context(tc.tile_pool(name="acc", bufs=2))
            ost_pool = bctx.enter_context(tc.tile_pool(name="ost", bufs=2))
            sps_pool = bctx.enter_context(
                tc.tile_pool(name="spsum", bufs=2, space="PSUM"))
            yps_pool = bctx.enter_context(
                tc.tile_pool(name="ypsum", bufs=3, space="PSUM"))
            rps = bctx.enter_context(
                tc.tile_pool(name="rpsum", bufs=1, space="PSUM"))

            state = [None] * NT_B

            def tile_hq(t):
                return t // NTQ, (t % NTQ) * TQ

            def open_tile(t):
                state[t] = {
                    "pbs": [None] * NKC,
                    "acc": acc_pool.tile([P, TQ], BF16, tag="acc", name="acc"),
                    "accr": red_pool.tile([P, TQ], BF16, tag="accr",
                                          name="accr"),
                    "yp": [yps_pool.tile([P, TQ], F32, tag="yp",
                                         name=f"yp{i}") for i in range(2)],
                    "s_row": None,
                }

            def emit_sps_pair(t, j):
                # two key chunks' score matmuls into one 2-bank PSUM tile,
                # drained by a single wide exp (halves ScalarE overhead)
                h, tq0 = tile_hq(t)
                st = state[t]
                sps = sps_pool.tile([P, 2, TQ], F32, tag="sps")
                pb = pb_pool.tile([P, 2, TQ], BF16, tag="pb")
                for u in range(2):
                    kc = 2 * j + u
                    for dc in range(2):
                        nc.tensor.matmul(
                            sps[:, u, :], kT[:, dc, kc * P:(kc + 1) * P],
                            qT[:, 2 * h + dc, tq0:tq0 + TQ],
                            start=(dc == 0), stop=(dc == 1))
                    st["pbs"][kc] = pb[:, u, :]
                nc.scalar.activation(pb[:], sps[:], EXP, scale=SCALE)

            def emit_yp(t, kc):
                st = state[t]
                for dc in range(2):
                    nc.tensor.matmul(
                        st["yp"][dc][:], v_sb[:, kc, dc * P:(dc + 1) * P],
                        st["pbs"][kc],
                        start=(kc == 0), stop=(kc == NKC - 1))

            def emit_add(t, kc):
                st = state[t]
                if kc == 1:
                    nc.vector.tensor_add(
                        st["acc"][:], st["pbs"][0], st["pbs"][1])
                elif kc == NKC - 1:
                    nc.vector.tensor_add(
                        st["accr"][:], st["acc"][:], st["pbs"][kc])
                else:
                    nc.vector.tensor_add(
                        st["acc"][:], st["acc"][:], st["pbs"][kc])

            def fin2a(t):
                # accr summed over its 128 key partitions AND broadcast to
                # all 128 output partitions in ONE all-ones matmul, then
                # fast-approx reciprocal
                st = state[t]
                emit_add(t, NKC - 1)
                bc_ps = rps.tile([P, TQ], F32, tag="r", name="bc_ps")
                nc.tensor.matmul(bc_ps[:], ones_mat[:], st["accr"][:],
                                 start=True, stop=True)
                rcp = red_pool.tile([P, TQ], F32, tag="rcp")
                nc.vector.reciprocal_approx_fast(rcp[:], bc_ps[:])
                st["rcp"] = rcp

            def fin2b(t):
                # normalization folded into the yp PSUM->SBUF drain
                st = state[t]
                h, tq0 = tile_hq(t)
                for dc in range(2):
                    nc.vector.tensor_mul(
                        yT[:, 2 * h + dc, tq0:tq0 + TQ],
                        st["yp"][dc][:], st["rcp"][:])
                state[t] = None

            for t in range(NT_B):
                open_tile(t)
                emit_sps_pair(t, 0)
                if t > 0:
                    fin2a(t - 1)
                    emit_yp(t - 1, NKC - 3)
                    emit_yp(t - 1, NKC - 2)
                    emit_yp(t - 1, NKC - 1)
                    fin2b(t - 1)
                for kc in range(2, NKC):
                    if kc % 2 == 0:
                        emit_sps_pair(t, kc // 2)
                    emit_add(t, kc - 1)
                    if kc >= 3:
                        emit_yp(t, kc - 3)

            # phase C tiles (reuse the sps PSUM banks; the last attention
            # finalize is sandwiched between the first C tiles)
            def emit_c(tch):
                ot = ost_pool.tile([P, C], BF16, tag="ot")
                for cop in range(2):
                    ps = sps_pool.tile([P, 2, TQ], F32, tag="sps", name="cps")
                    for u in range(2):
                        co = 2 * cop + u
                        for dc in range(FQ):
                            nc.tensor.matmul(
                                ps[:, u, :], yT[:, dc, tch * P:(tch + 1) * P],
                                wo_sb[:, dc, co * TQ:(co + 1) * TQ],
                                start=(dc == 0), stop=(dc == FQ - 1))
                        nc.scalar.copy(ot[:, co * TQ:(co + 1) * TQ],
                                       ps[:, u, :])
                nc.sync.dma_start(out[tch * P:(tch + 1) * P, :], ot[:])

            tl = NT_B - 1
            fin2a(tl)
            emit_yp(tl, NKC - 3)
            emit_yp(tl, NKC - 2)
            emit_yp(tl, NKC - 1)
            fin2b(tl)
            for tch in range(T // P):
                emit_c(tch)

    nc.compile()
    return nc


_NC = None
_TRACE = False      # set by test harness to capture an NTFF profile
_LAST_RES = None


def _get_nc():
    global _NC
    if _NC is None:
        _NC = build()
    return _NC


def kernel(x, position_ids, Wq, Wk, Wv, Wo):
    x = np.ascontiguousarray(np.asarray(x, dtype=np.float32))
    pos = np.asarray(position_ids)
    Wq = np.asarray(Wq, dtype=np.float32)
    Wk = np.asarray(Wk, dtype=np.float32)
    Wv = np.asarray(Wv, dtype=np.float32)
    Wo = np.asarray(Wo, dtype=np.float32)

    inv = 1.0 / (THETA ** (np.arange(0, D, 2, dtype=np.float64) / D))  # [128]

    in_maps = []
    for c in range(8):
        b, g = divmod(c, 4)
        xTb = np.ascontiguousarray(x[b].T).astype(ml_dtypes.bfloat16)  # [C,T]
        ang = inv[:, None] * pos[b].astype(np.float64)[None, :]        # [128,T]
        cosT = np.cos(ang).astype(np.float32)
        sinT = np.sin(ang).astype(np.float32)
        wqkv_np = np.ascontiguousarray(np.concatenate(
            [Wq[:, g * 1024:(g + 1) * 1024],
             Wk[:, g * 256:(g + 1) * 256],
             Wv[:, g * 256:(g + 1) * 256]], axis=1)).astype(ml_dtypes.bfloat16)
        wo_np = np.ascontiguousarray(
            Wo[g * 1024:(g + 1) * 1024, :]).astype(ml_dtypes.bfloat16)
        in_maps.append({"xT": xTb, "cosT": cosT, "sinT": sinT,
                        "wqkv": wqkv_np, "wo": wo_np})

    nc = _get_nc()
    res = run_bass_kernel_spmd(nc, in_maps, core_ids=list(range(8)),
                               trace=_TRACE)
    global _LAST_RES
    _LAST_RES = res
    outs = [r["out"].astype(np.float32) for r in res.results]
    return np.stack([outs[0] + outs[1] + outs[2] + outs[3],
                     outs[4] + outs[5] + outs[6] + outs[7]])


# revision 28
# speedup vs baseline: 1.0154x; 1.0035x over previous
# GemmaAttention on 8 Trainium2 NeuronCores — batch-DP x 4-head-TP.
#
# Sharding: core c -> batch b = c//4, head group g = c%4. Each core owns
# Q heads 4g..4g+3 (GQA group g exactly), KV head g, and tokens of batch
# b only. No redundant KV compute (38.7 GFLOP/core, the exact 1/8 of the
# model). Wq/Wk/Wv sliced column-wise, Wo row-wise; host sums 4 partials
# per batch. K and V stay SBUF-resident (no DRAM spill).
#
# Device kernel (per core, bf16 matmuls, f32 PSUM):
#   warmup) dummy matmuls overlap the initial weight/x DMA and keep the
#      HAM clock gate at 2.4 GHz from the first real matmul on.
#   A) QKV projection + RoPE: stream xT token tiles, accumulate q/k/v in
#      PSUM over the 2048 contraction, RoPE on drain (DVE). q/k/v all
#      stay in SBUF.
#   B) attention, software-pipelined ACROSS (head, query-tile) tiles:
#      the next tile's first two score groups are emitted before the
#      previous tile's trailing yP/finalize matmuls so the in-order PE
#      queue never waits on ScalarE exp at a tile boundary. Softmax
#      denominator: bf16 accumulate chain on DVE + rank-1 ones matmuls +
#      reciprocal_approx_fast; normalization folded into the yP
#      PSUM->SBUF drain (DVE).
#   C) output projection, emitted inside the B scope: reuses the score
#      PSUM banks, one row-wide [128, 2048] bf16 staging tile and a
#      single 512KB DMA per 128-token block; the last B finalize is
#      sandwiched between the first C tiles so nothing stalls.
import numpy as np
import ml_dtypes
from contextlib import ExitStack

import concourse.bass as bass
import concourse.mybir as mybir
import concourse.tile as tile
from concourse import bacc
from concourse.bass_utils import run_bass_kernel_spmd

P = 128
F32 = mybir.dt.float32
F32R = mybir.dt.float32r
BF16 = mybir.dt.bfloat16
EXP = mybir.ActivationFunctionType.Exp

B, T, C = 2, 2048, 2048
H, KV, D = 16, 4, 256
THETA = 10000.0
NH = H // KV            # 4 q heads per core
NCC = C // P            # 16 contraction chunks
TT_A = 512              # phase-A token tile
NT_A = T // TT_A        # 4
TQ = 512                # phase-B query tile
NTQ = T // TQ           # 4
NKC = T // P            # 16 key chunks
NT_B = NH * NTQ         # 16 attention tiles
SCALE = 1.0 / 16.0      # 1/sqrt(D)
FQ = 2 * NH             # 8 q feature chunks per core (4 heads x 256)
WCOLS = NH * D + 2 * D  # 1536 wqkv columns per core
NVC = T // P            # 16 v token chunks


def build():
    nc = bacc.Bacc("TRN2", target_bir_lowering=False, debug=False)
    xT = nc.dram_tensor("xT", [C, T], BF16, kind="ExternalInput").ap()
    cosT = nc.dram_tensor("cosT", [P, T], F32, kind="ExternalInput").ap()
    sinT = nc.dram_tensor("sinT", [P, T], F32, kind="ExternalInput").ap()
    wqkv = nc.dram_tensor("wqkv", [C, WCOLS], BF16, kind="ExternalInput").ap()
    wo = nc.dram_tensor("wo", [NH * D, C], BF16, kind="ExternalInput").ap()
    out = nc.dram_tensor("out", [T, C], BF16, kind="ExternalOutput").ap()

    xT3 = xT.rearrange("(o p) t -> p o t", p=P)        # [128, 16, 2048]
    wqkv3 = wqkv.rearrange("(o p) f -> p o f", p=P)    # [128, 16, 1536]
    wo3 = wo.rearrange("(o p) f -> p o f", p=P)        # [128, 8, 2048]

    with tile.TileContext(nc) as tc, ExitStack() as octx:
        const = octx.enter_context(tc.tile_pool(name="const", bufs=1))
        ones_mat = const.tile([P, P], BF16)
        nc.vector.memset(ones_mat[:], 1.0)
        wtile = const.tile([P, D], BF16)
        nc.vector.memset(wtile[:], 0.25)

        # resident across phases
        qres = octx.enter_context(tc.tile_pool(name="qres", bufs=1))
        qT = qres.tile([P, FQ, T], BF16)           # 32KB/part
        kT = qres.tile([P, 2, T], BF16)            # 8KB/part
        v_sb = qres.tile([P, NVC, D], BF16)        # 8KB/part

        # ---------------- Phase A: QKV projection + RoPE ----------------
        with ExitStack() as actx:
            wq_pool = actx.enter_context(tc.tile_pool(name="wq", bufs=1))
            wqkv_sb = wq_pool.tile([P, NCC, WCOLS], BF16)   # 48KB/part
            for cc in range(4):
                nc.sync.dma_start(wqkv_sb[:, cc, :], wqkv3[:, cc, :])

            xt_pool = actx.enter_context(tc.tile_pool(name="xt", bufs=2))
            cs_pool = actx.enter_context(tc.tile_pool(name="cs", bufs=2))
            tmp_pool = actx.enter_context(tc.tile_pool(name="tmp", bufs=2))
            aps = actx.enter_context(
                tc.tile_pool(name="apsum", bufs=4, space="PSUM"))
            vps_pool = actx.enter_context(
                tc.tile_pool(name="vpsum", bufs=2, space="PSUM"))
            wps = actx.enter_context(
                tc.tile_pool(name="wpsum", bufs=1, space="PSUM"))

            # warm the PE / HAM while the initial DMAs land (~13us)
            warm_ps = wps.tile([P, TQ], F32)
            for _ in range(112):
                nc.tensor.matmul(warm_ps[:, :D], wtile[:, :P], wtile[:],
                                 start=True, stop=True)

            for tt in range(NT_A):
                t0 = tt * TT_A
                xt = xt_pool.tile([P, NCC, TT_A], BF16, tag="xt")
                for g4 in range(4):
                    nc.sync.dma_start(
                        xt[:, 4 * g4:4 * g4 + 4, :],
                        xT3[:, 4 * g4:4 * g4 + 4, t0:t0 + TT_A])
                if tt == 0:
                    for cc in range(4, NCC):
                        nc.sync.dma_start(wqkv_sb[:, cc, :], wqkv3[:, cc, :])
                cos_t = cs_pool.tile([P, TT_A], F32, tag="cos")
                nc.sync.dma_start(cos_t[:], cosT[:, t0:t0 + TT_A])
                sin_t = cs_pool.tile([P, TT_A], F32, tag="sin")
                nc.sync.dma_start(sin_t[:], sinT[:, t0:t0 + TT_A])

                def rope(lo_ps, hi_ps, lo_out, hi_out):
                    # lo' = lo*cos - hi*sin ; hi' = hi*cos + lo*sin
                    ta = tmp_pool.tile([P, TT_A], F32, tag="ra")
                    tb = tmp_pool.tile([P, TT_A], F32, tag="rb")
                    nc.vector.tensor_mul(ta[:], lo_ps, cos_t[:])
                    nc.vector.tensor_mul(tb[:], hi_ps, sin_t[:])
                    nc.vector.tensor_sub(lo_out, ta[:], tb[:])
                    tc2 = tmp_pool.tile([P, TT_A], F32, tag="ra")
                    td = tmp_pool.tile([P, TT_A], F32, tag="rb")
                    nc.vector.tensor_mul(tc2[:], hi_ps, cos_t[:])
                    nc.vector.tensor_mul(td[:], lo_ps, sin_t[:])
                    nc.vector.tensor_add(hi_out, tc2[:], td[:])

                def q_pair(h):
                    qp = [None, None]
                    for dc in range(2):
                        j = 2 * h + dc
                        ps = aps.tile([P, TT_A], F32, tag="ps", name="qps")
                        for cc in range(NCC):
                            nc.tensor.matmul(
                                ps[:], wqkv_sb[:, cc, j * P:(j + 1) * P],
                                xt[:, cc, :],
                                start=(cc == 0), stop=(cc == NCC - 1))
                        qp[dc] = ps
                    rope(qp[0][:], qp[1][:],
                         qT[:, 2 * h, t0:t0 + TT_A],
                         qT[:, 2 * h + 1, t0:t0 + TT_A])

                def k_pair():
                    kp = [None, None]
                    for dc in range(2):
                        ps = aps.tile([P, TT_A], F32, tag="ps", name="kps")
                        for cc in range(NCC):
                            nc.tensor.matmul(
                                ps[:],
                                wqkv_sb[:, cc,
                                        NH * D + dc * P:NH * D + (dc + 1) * P],
                                xt[:, cc, :],
                                start=(cc == 0), stop=(cc == NCC - 1))
                        kp[dc] = ps
                    rope(kp[0][:], kp[1][:],
                         kT[:, 0, t0:t0 + TT_A], kT[:, 1, t0:t0 + TT_A])

                # last tile computes k first so kT is complete (rope done)
                # by the time phase B's first score matmuls need it
                if tt == NT_A - 1:
                    k_pair()
                for h in range(NH):
                    q_pair(h)
                if tt != NT_A - 1:
                    k_pair()

                # v: 4 token sub-chunks, no rope, straight into v_sb
                for s in range(4):
                    ps = vps_pool.tile([P, D], F32, tag="vps")
                    for cc in range(NCC):
                        nc.tensor.matmul(
                            ps[:], xt[:, cc, s * P:(s + 1) * P],
                            wqkv_sb[:, cc, NH * D + 2 * P:NH * D + 2 * P + D],
                            start=(cc == 0), stop=(cc == NCC - 1))
                    nc.scalar.copy(v_sb[:, 4 * tt + s, :], ps[:])

        # yT and wo_sb live in the address space wqkv_sb vacated
        with ExitStack() as bctx:
            yres = bctx.enter_context(tc.tile_pool(name="yres", bufs=1))
            yT = yres.tile([P, FQ, T], BF16)           # 32KB/part
            wo_sb = yres.tile([P, FQ, C], BF16)        # 32KB/part
            for dc in range(FQ):
                nc.sync.dma_start(wo_sb[:, dc, :], wo3[:, dc, :])

            # ------------- Phase B+C: attention + output projection -----
            pb_pool = bctx.enter_context(tc.tile_pool(name="pb", bufs=3))
            red_pool = bctx.enter_context(tc.tile_pool(name="red", bufs=2))
            acc_pool = bctx.enter_context(tc.tile_pool(name="acc", bufs=2))
            ost_pool = bctx.enter_context(tc.tile_pool(name="ost", bufs=2))
            sps_pool = bctx.enter_context(
                tc.tile_pool(name="spsum", bufs=2, space="PSUM"))
            yps_pool = bctx.enter_context(
                tc.tile_pool(name="ypsum", bufs=3, space="PSUM"))
            rps = bctx.enter_context(
                tc.tile_pool(name="rpsum", bufs=1, space="PSUM"))

            state = [None] * NT_B

            def tile_hq(t):
                return t // NTQ, (t % NTQ) * TQ

            def open_tile(t):
                state[t] = {
                    "pbs": [None] * NKC,
                    "acc": acc_pool.tile([P, TQ], BF16, tag="acc", name="acc"),
                    "accr": red_pool.tile([P, TQ], BF16, tag="accr",
                                          name="accr"),
                    "yp": [yps_pool.tile([P, TQ], F32, tag="yp",
                                         name=f"yp{i}") for i in range(2)],
                    "s_row": None,
                }

            def emit_sps_pair(t, j):
                # two key chunks' score matmuls into one 2-bank PSUM tile,
                # drained by a single wide exp (halves ScalarE overhead)
                h, tq0 = tile_hq(t)
                st = state[t]
                sps = sps_pool.tile([P, 2, TQ], F32, tag="sps")
                pb = pb_pool.tile([P, 2, TQ], BF16, tag="pb")
                for u in range(2):
                    kc = 2 * j + u
                    for dc in range(2):
                        nc.tensor.matmul(
                            sps[:, u, :], kT[:, dc, kc * P:(kc + 1) * P],
                            qT[:, 2 * h + dc, tq0:tq0 + TQ],
                            start=(dc == 0), stop=(dc == 1))
                    st["pbs"][kc] = pb[:, u, :]
                nc.scalar.activation(pb[:], sps[:], EXP, scale=SCALE)

            def emit_yp(t, kc):
                st = state[t]
                for dc in range(2):
                    nc.tensor.matmul(
                        st["yp"][dc][:], v_sb[:, kc, dc * P:(dc + 1) * P],
                        st["pbs"][kc],
                        start=(kc == 0), stop=(kc == NKC - 1))

            def emit_add(t, kc):
                st = state[t]
                if kc == 1:
                    nc.vector.tensor_add(
                        st["acc"][:], st["pbs"][0], st["pbs"][1])
                elif kc == NKC - 1:
                    nc.vector.tensor_add(
                        st["accr"][:], st["acc"][:], st["pbs"][kc])
                else:
                    nc.vector.tensor_add(
                        st["acc"][:], st["acc"][:], st["pbs"][kc])

            def fin2a(t):
                # accr summed over its 128 key partitions AND broadcast to
                # all 128 output partitions in ONE all-ones matmul, then
                # fast-approx reciprocal
                st = state[t]
                emit_add(t, NKC - 1)
                bc_ps = rps.tile([P, TQ], F32, tag="r", name="bc_ps")
                nc.tensor.matmul(bc_ps[:], ones_mat[:], st["accr"][:],
                                 start=True, stop=True)
                rcp = red_pool.tile([P, TQ], F32, tag="rcp")
                nc.vector.reciprocal_approx_fast(rcp[:], bc_ps[:])
                st["rcp"] = rcp

            def fin2b(t):
                # normalization folded into the yp PSUM->SBUF drain
                st = state[t]
                h, tq0 = tile_hq(t)
                for dc in range(2):
                    nc.vector.tensor_mul(
                        yT[:, 2 * h + dc, tq0:tq0 + TQ],
                        st["yp"][dc][:], st["rcp"][:])
                state[t] = None

            for t in range(NT_B):
                open_tile(t)
                emit_sps_pair(t, 0)
                if t > 0:
                    fin2a(t - 1)
                    emit_yp(t - 1, NKC - 3)
                    emit_yp(t - 1, NKC - 2)
                    emit_yp(t - 1, NKC - 1)
                    fin2b(t - 1)
                for kc in range(2, NKC):
                    if kc % 2 == 0:
                        emit_sps_pair(t, kc // 2)
                    emit_add(t, kc - 1)
                    if kc >= 3:
                        emit_yp(t, kc - 3)

            # phase C tiles (reuse the sps PSUM banks; the last attention
            # finalize is sandwiched between the first C tiles)
            def emit_c(tch):
                ot = ost_pool.tile([P, C], BF16, tag="ot")
                for cop in range(2):
                    ps = sps_pool.tile([P, 2, TQ], F32, tag="sps", name="cps")
                    for u in range(2):
                        co = 2 * cop + u
                        for dc in range(FQ):
                            nc.tensor.matmul(
                                ps[:, u, :], yT[:, dc, tch * P:(tch + 1) * P],
                                wo_sb[:, dc, co * TQ:(co + 1) * TQ],
                                start=(dc == 0), stop=(dc == FQ - 1))
                        nc.scalar.copy(ot[:, co * TQ:(co + 1) * TQ],
                                       ps[:, u, :])
                    nc.sync.dma_start(
                        out[tch * P:(tch + 1) * P,
                            cop * 2 * TQ:(cop + 1) * 2 * TQ],
                        ot[:, cop * 2 * TQ:(cop + 1) * 2 * TQ])

            tl = NT_B - 1
            fin2a(tl)
            emit_yp(tl, NKC - 3)
            emit_yp(tl, NKC - 2)
            emit_yp(tl, NKC - 1)
            fin2b(tl)
            for tch in range(T // P):
                emit_c(tch)

    nc.compile()
    return nc


_NC = None
_TRACE = False      # set by test harness to capture an NTFF profile
_LAST_RES = None


def _get_nc():
    global _NC
    if _NC is None:
        _NC = build()
    return _NC


def kernel(x, position_ids, Wq, Wk, Wv, Wo):
    x = np.ascontiguousarray(np.asarray(x, dtype=np.float32))
    pos = np.asarray(position_ids)
    Wq = np.asarray(Wq, dtype=np.float32)
    Wk = np.asarray(Wk, dtype=np.float32)
    Wv = np.asarray(Wv, dtype=np.float32)
    Wo = np.asarray(Wo, dtype=np.float32)

    inv = 1.0 / (THETA ** (np.arange(0, D, 2, dtype=np.float64) / D))  # [128]

    in_maps = []
    for c in range(8):
        b, g = divmod(c, 4)
        xTb = np.ascontiguousarray(x[b].T).astype(ml_dtypes.bfloat16)  # [C,T]
        ang = inv[:, None] * pos[b].astype(np.float64)[None, :]        # [128,T]
        cosT = np.cos(ang).astype(np.float32)
        sinT = np.sin(ang).astype(np.float32)
        wqkv_np = np.ascontiguousarray(np.concatenate(
            [Wq[:, g * 1024:(g + 1) * 1024],
             Wk[:, g * 256:(g + 1) * 256],
             Wv[:, g * 256:(g + 1) * 256]], axis=1)).astype(ml_dtypes.bfloat16)
        wo_np = np.ascontiguousarray(
            Wo[g * 1024:(g + 1) * 1024, :]).astype(ml_dtypes.bfloat16)
        in_maps.append({"xT": xTb, "cosT": cosT, "sinT": sinT,
                        "wqkv": wqkv_np, "wo": wo_np})

    nc = _get_nc()
    res = run_bass_kernel_spmd(nc, in_maps, core_ids=list(range(8)),
                               trace=_TRACE)
    global _LAST_RES
    _LAST_RES = res
    outs = [r["out"].astype(np.float32) for r in res.results]
    return np.stack([outs[0] + outs[1] + outs[2] + outs[3],
                     outs[4] + outs[5] + outs[6] + outs[7]])


# revision 30
# speedup vs baseline: 1.0288x; 1.0132x over previous
# GemmaAttention on 8 Trainium2 NeuronCores — batch-DP x 4-head-TP.
#
# Sharding: core c -> batch b = c//4, head group g = c%4. Each core owns
# Q heads 4g..4g+3 (GQA group g exactly), KV head g, and tokens of batch
# b only. No redundant KV compute (38.7 GFLOP/core, the exact 1/8 of the
# model). Wq/Wk/Wv sliced column-wise, Wo row-wise; host sums 4 partials
# per batch. K and V stay SBUF-resident (no DRAM spill).
#
# Device kernel (per core, bf16 matmuls, f32 PSUM):
#   warmup) dummy matmuls overlap the initial weight/x DMA and keep the
#      HAM clock gate at 2.4 GHz from the first real matmul on.
#   A) QKV projection + RoPE: stream xT token tiles, accumulate q/k/v in
#      PSUM over the 2048 contraction, RoPE on drain (DVE). q/k/v all
#      stay in SBUF.
#   B) attention, software-pipelined ACROSS (head, query-tile) tiles:
#      the next tile's first two score groups are emitted before the
#      previous tile's trailing yP/finalize matmuls so the in-order PE
#      queue never waits on ScalarE exp at a tile boundary. Softmax
#      denominator: bf16 accumulate chain on DVE + rank-1 ones matmuls +
#      reciprocal_approx_fast; normalization folded into the yP
#      PSUM->SBUF drain (DVE).
#   C) output projection, emitted inside the B scope: reuses the score
#      PSUM banks, one row-wide [128, 2048] bf16 staging tile and a
#      single 512KB DMA per 128-token block; the last B finalize is
#      sandwiched between the first C tiles so nothing stalls.
import numpy as np
import ml_dtypes
from contextlib import ExitStack

import concourse.bass as bass
import concourse.mybir as mybir
import concourse.tile as tile
from concourse import bacc
from concourse.bass_utils import run_bass_kernel_spmd

P = 128
F32 = mybir.dt.float32
F32R = mybir.dt.float32r
BF16 = mybir.dt.bfloat16
EXP = mybir.ActivationFunctionType.Exp

B, T, C = 2, 2048, 2048
H, KV, D = 16, 4, 256
THETA = 10000.0
NH = H // KV            # 4 q heads per core
NCC = C // P            # 16 contraction chunks
TT_A = 512              # phase-A token tile
NT_A = T // TT_A        # 4
TQ = 512                # phase-B query tile
NTQ = T // TQ           # 4
NKC = T // P            # 16 key chunks
NT_B = NH * NTQ         # 16 attention tiles
SCALE = 1.0 / 16.0      # 1/sqrt(D)
FQ = 2 * NH             # 8 q feature chunks per core (4 heads x 256)
WCOLS = NH * D + 2 * D  # 1536 wqkv columns per core
NVC = T // P            # 16 v token chunks


def build():
    nc = bacc.Bacc("TRN2", target_bir_lowering=False, debug=False)
    xT = nc.dram_tensor("xT", [C, T], BF16, kind="ExternalInput").ap()
    cosT = nc.dram_tensor("cosT", [P, T], F32, kind="ExternalInput").ap()
    sinT = nc.dram_tensor("sinT", [P, T], F32, kind="ExternalInput").ap()
    wqkv = nc.dram_tensor("wqkv", [C, WCOLS], BF16, kind="ExternalInput").ap()
    wo = nc.dram_tensor("wo", [NH * D, C], BF16, kind="ExternalInput").ap()
    out = nc.dram_tensor("out", [T, C], BF16, kind="ExternalOutput").ap()

    xT3 = xT.rearrange("(o p) t -> p o t", p=P)        # [128, 16, 2048]
    wqkv3 = wqkv.rearrange("(o p) f -> p o f", p=P)    # [128, 16, 1536]
    wo3 = wo.rearrange("(o p) f -> p o f", p=P)        # [128, 8, 2048]

    with tile.TileContext(nc) as tc, ExitStack() as octx:
        const = octx.enter_context(tc.tile_pool(name="const", bufs=1))
        ones_mat = const.tile([P, P], BF16)
        nc.vector.memset(ones_mat[:], 1.0)
        wtile = const.tile([P, D], BF16)
        nc.vector.memset(wtile[:], 0.25)

        # resident across phases
        qres = octx.enter_context(tc.tile_pool(name="qres", bufs=1))
        qT = qres.tile([P, FQ, T], BF16)           # 32KB/part
        kT = qres.tile([P, 2, T], BF16)            # 8KB/part
        v_sb = qres.tile([P, NVC, D], BF16)        # 8KB/part

        # ---------------- Phase A: QKV projection + RoPE ----------------
        with ExitStack() as actx:
            wq_pool = actx.enter_context(tc.tile_pool(name="wq", bufs=1))
            wqkv_sb = wq_pool.tile([P, NCC, WCOLS], BF16)   # 48KB/part
            for cc in range(4):
                nc.sync.dma_start(wqkv_sb[:, cc, :], wqkv3[:, cc, :])

            xt_pool = actx.enter_context(tc.tile_pool(name="xt", bufs=2))
            cs_pool = actx.enter_context(tc.tile_pool(name="cs", bufs=2))
            tmp_pool = actx.enter_context(tc.tile_pool(name="tmp", bufs=2))
            aps = actx.enter_context(
                tc.tile_pool(name="apsum", bufs=4, space="PSUM"))
            vps_pool = actx.enter_context(
                tc.tile_pool(name="vpsum", bufs=2, space="PSUM"))
            wps = actx.enter_context(
                tc.tile_pool(name="wpsum", bufs=1, space="PSUM"))

            # warm the PE / HAM while the initial DMAs land (~8us)
            warm_ps = wps.tile([P, TQ], F32)
            for _ in range(52):
                nc.tensor.matmul(warm_ps[:, :D], wtile[:, :P], wtile[:],
                                 start=True, stop=True)

            for tt in range(NT_A):
                t0 = tt * TT_A
                xt = xt_pool.tile([P, NCC, TT_A], BF16, tag="xt")
                for g4 in range(4):
                    nc.sync.dma_start(
                        xt[:, 4 * g4:4 * g4 + 4, :],
                        xT3[:, 4 * g4:4 * g4 + 4, t0:t0 + TT_A])
                if tt == 0:
                    for cc in range(4, NCC):
                        nc.sync.dma_start(wqkv_sb[:, cc, :], wqkv3[:, cc, :])
                cos_t = cs_pool.tile([P, TT_A], F32, tag="cos")
                nc.sync.dma_start(cos_t[:], cosT[:, t0:t0 + TT_A])
                sin_t = cs_pool.tile([P, TT_A], F32, tag="sin")
                nc.sync.dma_start(sin_t[:], sinT[:, t0:t0 + TT_A])

                def rope(lo_ps, hi_ps, lo_out, hi_out):
                    # lo' = lo*cos - hi*sin ; hi' = hi*cos + lo*sin
                    ta = tmp_pool.tile([P, TT_A], F32, tag="ra")
                    tb = tmp_pool.tile([P, TT_A], F32, tag="rb")
                    nc.vector.tensor_mul(ta[:], lo_ps, cos_t[:])
                    nc.vector.tensor_mul(tb[:], hi_ps, sin_t[:])
                    nc.vector.tensor_sub(lo_out, ta[:], tb[:])
                    tc2 = tmp_pool.tile([P, TT_A], F32, tag="ra")
                    td = tmp_pool.tile([P, TT_A], F32, tag="rb")
                    nc.vector.tensor_mul(tc2[:], hi_ps, cos_t[:])
                    nc.vector.tensor_mul(td[:], lo_ps, sin_t[:])
                    nc.vector.tensor_add(hi_out, tc2[:], td[:])

                def q_pair(h):
                    qp = [None, None]
                    for dc in range(2):
                        j = 2 * h + dc
                        ps = aps.tile([P, TT_A], F32, tag="ps", name="qps")
                        for cc in range(NCC):
                            nc.tensor.matmul(
                                ps[:], wqkv_sb[:, cc, j * P:(j + 1) * P],
                                xt[:, cc, :],
                                start=(cc == 0), stop=(cc == NCC - 1))
                        qp[dc] = ps
                    rope(qp[0][:], qp[1][:],
                         qT[:, 2 * h, t0:t0 + TT_A],
                         qT[:, 2 * h + 1, t0:t0 + TT_A])

                def k_pair():
                    kp = [None, None]
                    for dc in range(2):
                        ps = aps.tile([P, TT_A], F32, tag="ps", name="kps")
                        for cc in range(NCC):
                            nc.tensor.matmul(
                                ps[:],
                                wqkv_sb[:, cc,
                                        NH * D + dc * P:NH * D + (dc + 1) * P],
                                xt[:, cc, :],
                                start=(cc == 0), stop=(cc == NCC - 1))
                        kp[dc] = ps
                    rope(kp[0][:], kp[1][:],
                         kT[:, 0, t0:t0 + TT_A], kT[:, 1, t0:t0 + TT_A])

                # last tile computes k first so kT is complete (rope done)
                # by the time phase B's first score matmuls need it
                if tt == NT_A - 1:
                    k_pair()
                for h in range(NH):
                    q_pair(h)
                if tt != NT_A - 1:
                    k_pair()

                # v: 4 token sub-chunks, no rope, straight into v_sb
                for s in range(4):
                    ps = vps_pool.tile([P, D], F32, tag="vps")
                    for cc in range(NCC):
                        nc.tensor.matmul(
                            ps[:], xt[:, cc, s * P:(s + 1) * P],
                            wqkv_sb[:, cc, NH * D + 2 * P:NH * D + 2 * P + D],
                            start=(cc == 0), stop=(cc == NCC - 1))
                    nc.scalar.copy(v_sb[:, 4 * tt + s, :], ps[:])

        # yT and wo_sb live in the address space wqkv_sb vacated
        with ExitStack() as bctx:
            yres = bctx.enter_context(tc.tile_pool(name="yres", bufs=1))
            yT = yres.tile([P, FQ, T], BF16)           # 32KB/part
            wo_sb = yres.tile([P, FQ, C], BF16)        # 32KB/part
            for dc in range(FQ):
                nc.sync.dma_start(wo_sb[:, dc, :], wo3[:, dc, :])

            # ------------- Phase B+C: attention + output projection -----
            pb_pool = bctx.enter_context(tc.tile_pool(name="pb", bufs=3))
            red_pool = bctx.enter_context(tc.tile_pool(name="red", bufs=2))
            acc_pool = bctx.enter_context(tc.tile_pool(name="acc", bufs=2))
            ost_pool = bctx.enter_context(tc.tile_pool(name="ost", bufs=2))
            sps_pool = bctx.enter_context(
                tc.tile_pool(name="spsum", bufs=2, space="PSUM"))
            yps_pool = bctx.enter_context(
                tc.tile_pool(name="ypsum", bufs=3, space="PSUM"))
            rps = bctx.enter_context(
                tc.tile_pool(name="rpsum", bufs=1, space="PSUM"))

            state = [None] * NT_B

            def tile_hq(t):
                return t // NTQ, (t % NTQ) * TQ

            def open_tile(t):
                state[t] = {
                    "pbs": [None] * NKC,
                    "acc": acc_pool.tile([P, TQ], BF16, tag="acc", name="acc"),
                    "accr": red_pool.tile([P, TQ], BF16, tag="accr",
                                          name="accr"),
                    "yp": [yps_pool.tile([P, TQ], F32, tag="yp",
                                         name=f"yp{i}") for i in range(2)],
                    "s_row": None,
                }

            def emit_sps_pair(t, j):
                # two key chunks' score matmuls into one 2-bank PSUM tile,
                # drained by a single wide exp (halves ScalarE overhead)
                h, tq0 = tile_hq(t)
                st = state[t]
                sps = sps_pool.tile([P, 2, TQ], F32, tag="sps")
                pb = pb_pool.tile([P, 2, TQ], BF16, tag="pb")
                for u in range(2):
                    kc = 2 * j + u
                    for dc in range(2):
                        nc.tensor.matmul(
                            sps[:, u, :], kT[:, dc, kc * P:(kc + 1) * P],
                            qT[:, 2 * h + dc, tq0:tq0 + TQ],
                            start=(dc == 0), stop=(dc == 1))
                    st["pbs"][kc] = pb[:, u, :]
                nc.scalar.activation(pb[:], sps[:], EXP, scale=SCALE)

            def emit_yp(t, kc):
                st = state[t]
                for dc in range(2):
                    nc.tensor.matmul(
                        st["yp"][dc][:], v_sb[:, kc, dc * P:(dc + 1) * P],
                        st["pbs"][kc],
                        start=(kc == 0), stop=(kc == NKC - 1))

            def emit_add(t, kc):
                st = state[t]
                if kc == 1:
                    nc.vector.tensor_add(
                        st["acc"][:], st["pbs"][0], st["pbs"][1])
                elif kc == NKC - 1:
                    nc.vector.tensor_add(
                        st["accr"][:], st["acc"][:], st["pbs"][kc])
                else:
                    nc.vector.tensor_add(
                        st["acc"][:], st["acc"][:], st["pbs"][kc])

            def fin2a(t):
                # accr summed over its 128 key partitions AND broadcast to
                # all 128 output partitions in ONE all-ones matmul, then
                # fast-approx reciprocal
                st = state[t]
                emit_add(t, NKC - 1)
                bc_ps = rps.tile([P, TQ], F32, tag="r", name="bc_ps")
                nc.tensor.matmul(bc_ps[:], ones_mat[:], st["accr"][:],
                                 start=True, stop=True)
                rcp = red_pool.tile([P, TQ], F32, tag="rcp")
                nc.vector.reciprocal_approx_fast(rcp[:], bc_ps[:])
                st["rcp"] = rcp

            def fin2b(t):
                # normalization folded into the yp PSUM->SBUF drain
                st = state[t]
                h, tq0 = tile_hq(t)
                for dc in range(2):
                    nc.vector.tensor_mul(
                        yT[:, 2 * h + dc, tq0:tq0 + TQ],
                        st["yp"][dc][:], st["rcp"][:])
                state[t] = None

            for t in range(NT_B):
                open_tile(t)
                emit_sps_pair(t, 0)
                if t > 0:
                    fin2a(t - 1)
                    emit_yp(t - 1, NKC - 3)
                    emit_yp(t - 1, NKC - 2)
                    emit_yp(t - 1, NKC - 1)
                    fin2b(t - 1)
                for kc in range(2, NKC):
                    if kc % 2 == 0:
                        emit_sps_pair(t, kc // 2)
                    emit_add(t, kc - 1)
                    if kc >= 3:
                        emit_yp(t, kc - 3)

            # phase C tiles (reuse the sps PSUM banks; the last attention
            # finalize is sandwiched between the first C tiles)
            def emit_c(tch):
                ot = ost_pool.tile([P, C], BF16, tag="ot")
                for co in range(C // TQ):
                    ps = yps_pool.tile([P, TQ], F32, tag="yp", name="cps")
                    for dc in range(FQ):
                        nc.tensor.matmul(
                            ps[:], yT[:, dc, tch * P:(tch + 1) * P],
                            wo_sb[:, dc, co * TQ:(co + 1) * TQ],
                            start=(dc == 0), stop=(dc == FQ - 1))
                    nc.scalar.copy(ot[:, co * TQ:(co + 1) * TQ], ps[:])
                    if co % 2 == 1:
                        nc.sync.dma_start(
                            out[tch * P:(tch + 1) * P,
                                (co - 1) * TQ:(co + 1) * TQ],
                            ot[:, (co - 1) * TQ:(co + 1) * TQ])

            tl = NT_B - 1
            fin2a(tl)
            emit_yp(tl, NKC - 3)
            emit_yp(tl, NKC - 2)
            emit_yp(tl, NKC - 1)
            fin2b(tl)
            for tch in range(T // P):
                emit_c(tch)

    nc.compile()
    return nc


_NC = None
_TRACE = False      # set by test harness to capture an NTFF profile
_LAST_RES = None


def _get_nc():
    global _NC
    if _NC is None:
        _NC = build()
    return _NC


def kernel(x, position_ids, Wq, Wk, Wv, Wo):
    x = np.ascontiguousarray(np.asarray(x, dtype=np.float32))
    pos = np.asarray(position_ids)
    Wq = np.asarray(Wq, dtype=np.float32)
    Wk = np.asarray(Wk, dtype=np.float32)
    Wv = np.asarray(Wv, dtype=np.float32)
    Wo = np.asarray(Wo, dtype=np.float32)

    inv = 1.0 / (THETA ** (np.arange(0, D, 2, dtype=np.float64) / D))  # [128]

    in_maps = []
    for c in range(8):
        b, g = divmod(c, 4)
        xTb = np.ascontiguousarray(x[b].T).astype(ml_dtypes.bfloat16)  # [C,T]
        ang = inv[:, None] * pos[b].astype(np.float64)[None, :]        # [128,T]
        cosT = np.cos(ang).astype(np.float32)
        sinT = np.sin(ang).astype(np.float32)
        wqkv_np = np.ascontiguousarray(np.concatenate(
            [Wq[:, g * 1024:(g + 1) * 1024],
             Wk[:, g * 256:(g + 1) * 256],
             Wv[:, g * 256:(g + 1) * 256]], axis=1)).astype(ml_dtypes.bfloat16)
        wo_np = np.ascontiguousarray(
            Wo[g * 1024:(g + 1) * 1024, :]).astype(ml_dtypes.bfloat16)
        in_maps.append({"xT": xTb, "cosT": cosT, "sinT": sinT,
                        "wqkv": wqkv_np, "wo": wo_np})

    nc = _get_nc()
    res = run_bass_kernel_spmd(nc, in_maps, core_ids=list(range(8)),
                               trace=_TRACE)
    global _LAST_RES
    _LAST_RES = res
    outs = [r["out"].astype(np.float32) for r in res.results]
    return np.stack([outs[0] + outs[1] + outs[2] + outs[3],
                     outs[4] + outs[5] + outs[6] + outs[7]])


# revision 36
# speedup vs baseline: 1.0295x; 1.0006x over previous
# GemmaAttention on 8 Trainium2 NeuronCores — batch-DP x 4-head-TP.
#
# Sharding: core c -> batch b = c//4, head group g = c%4. Each core owns
# Q heads 4g..4g+3 (GQA group g exactly), KV head g, and tokens of batch
# b only. No redundant KV compute (38.7 GFLOP/core, the exact 1/8 of the
# model). Wq/Wk/Wv sliced column-wise, Wo row-wise; host sums 4 partials
# per batch. K and V stay SBUF-resident (no DRAM spill).
#
# Device kernel (per core, bf16 matmuls, f32 PSUM):
#   warmup) dummy matmuls overlap the initial weight/x DMA and keep the
#      HAM clock gate at 2.4 GHz from the first real matmul on.
#   A) QKV projection + RoPE: stream xT token tiles, accumulate q/k/v in
#      PSUM over the 2048 contraction, RoPE on drain (DVE). q/k/v all
#      stay in SBUF.
#   B) attention, software-pipelined ACROSS (head, query-tile) tiles:
#      the next tile's first two score groups are emitted before the
#      previous tile's trailing yP/finalize matmuls so the in-order PE
#      queue never waits on ScalarE exp at a tile boundary. Softmax
#      denominator: bf16 accumulate chain on DVE + rank-1 ones matmuls +
#      reciprocal_approx_fast; normalization folded into the yP
#      PSUM->SBUF drain (DVE).
#   C) output projection, emitted inside the B scope: reuses the score
#      PSUM banks, one row-wide [128, 2048] bf16 staging tile and a
#      single 512KB DMA per 128-token block; the last B finalize is
#      sandwiched between the first C tiles so nothing stalls.
import numpy as np
import ml_dtypes
from contextlib import ExitStack

import concourse.bass as bass
import concourse.mybir as mybir
import concourse.tile as tile
from concourse import bacc
from concourse.bass_utils import run_bass_kernel_spmd

P = 128
F32 = mybir.dt.float32
F32R = mybir.dt.float32r
BF16 = mybir.dt.bfloat16
EXP = mybir.ActivationFunctionType.Exp

B, T, C = 2, 2048, 2048
H, KV, D = 16, 4, 256
THETA = 10000.0
NH = H // KV            # 4 q heads per core
NCC = C // P            # 16 contraction chunks
TT_A = 512              # phase-A token tile
NT_A = T // TT_A        # 4
TQ = 512                # phase-B query tile
NTQ = T // TQ           # 4
NKC = T // P            # 16 key chunks
NT_B = NH * NTQ         # 16 attention tiles
SCALE = 1.0 / 16.0      # 1/sqrt(D)
FQ = 2 * NH             # 8 q feature chunks per core (4 heads x 256)
WCOLS = NH * D + 2 * D  # 1536 wqkv columns per core
NVC = T // P            # 16 v token chunks


def build():
    nc = bacc.Bacc("TRN2", target_bir_lowering=False, debug=False)
    # x pre-tiled host-side to [tile, partition, chunk, token] so each
    # phase-A DMA reads 4KB+ contiguous per partition (big DMA packets)
    xT = nc.dram_tensor("xT", [NT_A, P, NCC, TT_A], BF16,
                        kind="ExternalInput").ap()
    cosT = nc.dram_tensor("cosT", [P, T], F32, kind="ExternalInput").ap()
    sinT = nc.dram_tensor("sinT", [P, T], F32, kind="ExternalInput").ap()
    wqkv = nc.dram_tensor("wqkv", [C, WCOLS], BF16, kind="ExternalInput").ap()
    wo = nc.dram_tensor("wo", [NH * D, C], BF16, kind="ExternalInput").ap()
    out = nc.dram_tensor("out", [T, C], BF16, kind="ExternalOutput").ap()

    wqkv3 = wqkv.rearrange("(o p) f -> p o f", p=P)    # [128, 16, 1536]
    wo3 = wo.rearrange("(o p) f -> p o f", p=P)        # [128, 8, 2048]

    with tile.TileContext(nc) as tc, ExitStack() as octx:
        const = octx.enter_context(tc.tile_pool(name="const", bufs=1))
        ones_mat = const.tile([P, P], BF16)
        nc.vector.memset(ones_mat[:], 1.0)
        wtile = const.tile([P, D], BF16)
        nc.vector.memset(wtile[:], 0.25)

        # resident across phases
        qres = octx.enter_context(tc.tile_pool(name="qres", bufs=1))
        qT = qres.tile([P, FQ, T], BF16)           # 32KB/part
        kT = qres.tile([P, 2, T], BF16)            # 8KB/part
        v_sb = qres.tile([P, NVC, D], BF16)        # 8KB/part

        # ---------------- Phase A: QKV projection + RoPE ----------------
        with ExitStack() as actx:
            wq_pool = actx.enter_context(tc.tile_pool(name="wq", bufs=1))
            wqkv_sb = wq_pool.tile([P, NCC, WCOLS], BF16)   # 48KB/part
            for cc in range(4):
                nc.sync.dma_start(wqkv_sb[:, cc, :], wqkv3[:, cc, :])

            xt_pool = actx.enter_context(tc.tile_pool(name="xt", bufs=2))
            cs_pool = actx.enter_context(tc.tile_pool(name="cs", bufs=2))
            tmp_pool = actx.enter_context(tc.tile_pool(name="tmp", bufs=2))
            aps = actx.enter_context(
                tc.tile_pool(name="apsum", bufs=4, space="PSUM"))
            vps_pool = actx.enter_context(
                tc.tile_pool(name="vpsum", bufs=2, space="PSUM"))
            wps = actx.enter_context(
                tc.tile_pool(name="wpsum", bufs=1, space="PSUM"))

            # warm the PE / HAM while the initial DMAs land (~10us)
            warm_ps = wps.tile([P, TQ], F32)
            for _ in range(72):
                nc.tensor.matmul(warm_ps[:, :D], wtile[:, :P], wtile[:],
                                 start=True, stop=True)

            for tt in range(NT_A):
                t0 = tt * TT_A
                xt = xt_pool.tile([P, NCC, TT_A], BF16, tag="xt")
                for g4 in range(4):
                    nc.sync.dma_start(
                        xt[:, 4 * g4:4 * g4 + 4, :],
                        xT[tt, :, 4 * g4:4 * g4 + 4, :])
                if tt == 0:
                    for cc in range(4, NCC):
                        nc.sync.dma_start(wqkv_sb[:, cc, :], wqkv3[:, cc, :])
                cos_t = cs_pool.tile([P, TT_A], F32, tag="cos")
                nc.sync.dma_start(cos_t[:], cosT[:, t0:t0 + TT_A])
                sin_t = cs_pool.tile([P, TT_A], F32, tag="sin")
                nc.sync.dma_start(sin_t[:], sinT[:, t0:t0 + TT_A])

                def rope(lo_ps, hi_ps, lo_out, hi_out):
                    # lo' = lo*cos - hi*sin ; hi' = hi*cos + lo*sin
                    ta = tmp_pool.tile([P, TT_A], F32, tag="ra")
                    tb = tmp_pool.tile([P, TT_A], F32, tag="rb")
                    nc.vector.tensor_mul(ta[:], lo_ps, cos_t[:])
                    nc.vector.tensor_mul(tb[:], hi_ps, sin_t[:])
                    nc.vector.tensor_sub(lo_out, ta[:], tb[:])
                    tc2 = tmp_pool.tile([P, TT_A], F32, tag="ra")
                    td = tmp_pool.tile([P, TT_A], F32, tag="rb")
                    nc.vector.tensor_mul(tc2[:], hi_ps, cos_t[:])
                    nc.vector.tensor_mul(td[:], lo_ps, sin_t[:])
                    nc.vector.tensor_add(hi_out, tc2[:], td[:])

                def q_pair(h):
                    qp = [None, None]
                    for dc in range(2):
                        j = 2 * h + dc
                        ps = aps.tile([P, TT_A], F32, tag="ps", name="qps")
                        for cc in range(NCC):
                            nc.tensor.matmul(
                                ps[:], wqkv_sb[:, cc, j * P:(j + 1) * P],
                                xt[:, cc, :],
                                start=(cc == 0), stop=(cc == NCC - 1))
                        qp[dc] = ps
                    rope(qp[0][:], qp[1][:],
                         qT[:, 2 * h, t0:t0 + TT_A],
                         qT[:, 2 * h + 1, t0:t0 + TT_A])

                def k_pair():
                    kp = [None, None]
                    for dc in range(2):
                        ps = aps.tile([P, TT_A], F32, tag="ps", name="kps")
                        for cc in range(NCC):
                            nc.tensor.matmul(
                                ps[:],
                                wqkv_sb[:, cc,
                                        NH * D + dc * P:NH * D + (dc + 1) * P],
                                xt[:, cc, :],
                                start=(cc == 0), stop=(cc == NCC - 1))
                        kp[dc] = ps
                    rope(kp[0][:], kp[1][:],
                         kT[:, 0, t0:t0 + TT_A], kT[:, 1, t0:t0 + TT_A])

                # last tile computes k first so kT is complete (rope done)
                # by the time phase B's first score matmuls need it
                if tt == NT_A - 1:
                    k_pair()
                for h in range(NH):
                    q_pair(h)
                if tt != NT_A - 1:
                    k_pair()

                # v: 4 token sub-chunks, no rope, straight into v_sb
                for s in range(4):
                    ps = vps_pool.tile([P, D], F32, tag="vps")
                    for cc in range(NCC):
                        nc.tensor.matmul(
                            ps[:], xt[:, cc, s * P:(s + 1) * P],
                            wqkv_sb[:, cc, NH * D + 2 * P:NH * D + 2 * P + D],
                            start=(cc == 0), stop=(cc == NCC - 1))
                    nc.scalar.copy(v_sb[:, 4 * tt + s, :], ps[:])

        # yT and wo_sb live in the address space wqkv_sb vacated
        with ExitStack() as bctx:
            yres = bctx.enter_context(tc.tile_pool(name="yres", bufs=1))
            yT = yres.tile([P, FQ, T], BF16)           # 32KB/part
            wo_sb = yres.tile([P, FQ, C], BF16)        # 32KB/part
            for dc in range(FQ):
                nc.sync.dma_start(wo_sb[:, dc, :], wo3[:, dc, :])

            # ------------- Phase B+C: attention + output projection -----
            pb_pool = bctx.enter_context(tc.tile_pool(name="pb", bufs=3))
            red_pool = bctx.enter_context(tc.tile_pool(name="red", bufs=2))
            acc_pool = bctx.enter_context(tc.tile_pool(name="acc", bufs=2))
            ost_pool = bctx.enter_context(tc.tile_pool(name="ost", bufs=3))
            sps_pool = bctx.enter_context(
                tc.tile_pool(name="spsum", bufs=2, space="PSUM"))
            yps_pool = bctx.enter_context(
                tc.tile_pool(name="ypsum", bufs=3, space="PSUM"))
            rps = bctx.enter_context(
                tc.tile_pool(name="rpsum", bufs=1, space="PSUM"))

            state = [None] * NT_B

            def tile_hq(t):
                return t // NTQ, (t % NTQ) * TQ

            def open_tile(t):
                state[t] = {
                    "pbs": [None] * NKC,
                    "acc": acc_pool.tile([P, TQ], BF16, tag="acc", name="acc"),
                    "accr": red_pool.tile([P, TQ], BF16, tag="accr",
                                          name="accr"),
                    "yp": [yps_pool.tile([P, TQ], F32, tag="yp",
                                         name=f"yp{i}") for i in range(2)],
                    "s_row": None,
                }

            def emit_sps_pair(t, j):
                # two key chunks' score matmuls into one 2-bank PSUM tile,
                # drained by a single wide exp (halves ScalarE overhead)
                h, tq0 = tile_hq(t)
                st = state[t]
                sps = sps_pool.tile([P, 2, TQ], F32, tag="sps")
                pb = pb_pool.tile([P, 2, TQ], BF16, tag="pb")
                for u in range(2):
                    kc = 2 * j + u
                    for dc in range(2):
                        nc.tensor.matmul(
                            sps[:, u, :], kT[:, dc, kc * P:(kc + 1) * P],
                            qT[:, 2 * h + dc, tq0:tq0 + TQ],
                            start=(dc == 0), stop=(dc == 1))
                    st["pbs"][kc] = pb[:, u, :]
                nc.scalar.activation(pb[:], sps[:], EXP, scale=SCALE)

            def emit_yp(t, kc):
                st = state[t]
                for dc in range(2):
                    nc.tensor.matmul(
                        st["yp"][dc][:], v_sb[:, kc, dc * P:(dc + 1) * P],
                        st["pbs"][kc],
                        start=(kc == 0), stop=(kc == NKC - 1))

            def emit_add(t, kc):
                st = state[t]
                if kc == 1:
                    nc.vector.tensor_add(
                        st["acc"][:], st["pbs"][0], st["pbs"][1])
                elif kc == NKC - 1:
                    nc.vector.tensor_add(
                        st["accr"][:], st["acc"][:], st["pbs"][kc])
                else:
                    nc.vector.tensor_add(
                        st["acc"][:], st["acc"][:], st["pbs"][kc])

            def fin2a(t):
                # accr summed over its 128 key partitions AND broadcast to
                # all 128 output partitions in ONE all-ones matmul, then
                # fast-approx reciprocal
                st = state[t]
                emit_add(t, NKC - 1)
                bc_ps = rps.tile([P, TQ], F32, tag="r", name="bc_ps")
                nc.tensor.matmul(bc_ps[:], ones_mat[:], st["accr"][:],
                                 start=True, stop=True)
                rcp = red_pool.tile([P, TQ], F32, tag="rcp")
                nc.vector.reciprocal_approx_fast(rcp[:], bc_ps[:])
                st["rcp"] = rcp

            def fin2b(t):
                # normalization folded into the yp PSUM->SBUF drain
                st = state[t]
                h, tq0 = tile_hq(t)
                for dc in range(2):
                    nc.vector.tensor_mul(
                        yT[:, 2 * h + dc, tq0:tq0 + TQ],
                        st["yp"][dc][:], st["rcp"][:])
                state[t] = None

            for t in range(NT_B):
                open_tile(t)
                emit_sps_pair(t, 0)
                if t > 0:
                    fin2a(t - 1)
                    emit_yp(t - 1, NKC - 3)
                    emit_yp(t - 1, NKC - 2)
                    emit_yp(t - 1, NKC - 1)
                    fin2b(t - 1)
                for kc in range(2, NKC):
                    if kc % 2 == 0:
                        emit_sps_pair(t, kc // 2)
                    emit_add(t, kc - 1)
                    if kc >= 3:
                        emit_yp(t, kc - 3)

            # phase C tiles (reuse the sps PSUM banks; the last attention
            # finalize is sandwiched between the first C tiles)
            def emit_c(tch):
                ot = ost_pool.tile([P, C], BF16, tag="ot")
                for co in range(C // TQ):
                    ps = yps_pool.tile([P, TQ], F32, tag="yp", name="cps")
                    for dc in range(FQ):
                        nc.tensor.matmul(
                            ps[:], yT[:, dc, tch * P:(tch + 1) * P],
                            wo_sb[:, dc, co * TQ:(co + 1) * TQ],
                            start=(dc == 0), stop=(dc == FQ - 1))
                    nc.scalar.copy(ot[:, co * TQ:(co + 1) * TQ], ps[:])
                    if co % 2 == 1:
                        nc.sync.dma_start(
                            out[tch * P:(tch + 1) * P,
                                (co - 1) * TQ:(co + 1) * TQ],
                            ot[:, (co - 1) * TQ:(co + 1) * TQ])

            tl = NT_B - 1
            fin2a(tl)
            emit_yp(tl, NKC - 3)
            emit_yp(tl, NKC - 2)
            emit_yp(tl, NKC - 1)
            fin2b(tl)
            for tch in range(T // P):
                emit_c(tch)

    nc.compile()
    return nc


_NC = None
_TRACE = False      # set by test harness to capture an NTFF profile
_LAST_RES = None


def _get_nc():
    global _NC
    if _NC is None:
        _NC = build()
    return _NC


def kernel(x, position_ids, Wq, Wk, Wv, Wo):
    x = np.ascontiguousarray(np.asarray(x, dtype=np.float32))
    pos = np.asarray(position_ids)
    Wq = np.asarray(Wq, dtype=np.float32)
    Wk = np.asarray(Wk, dtype=np.float32)
    Wv = np.asarray(Wv, dtype=np.float32)
    Wo = np.asarray(Wo, dtype=np.float32)

    inv = 1.0 / (THETA ** (np.arange(0, D, 2, dtype=np.float64) / D))  # [128]

    in_maps = []
    for c in range(8):
        b, g = divmod(c, 4)
        # [NT_A, P, NCC, TT_A]: per-partition-contiguous phase-A tiles
        xTb = np.ascontiguousarray(
            x[b].T.reshape(NCC, P, NT_A, TT_A).transpose(2, 1, 0, 3)
        ).astype(ml_dtypes.bfloat16)
        ang = inv[:, None] * pos[b].astype(np.float64)[None, :]        # [128,T]
        cosT = np.cos(ang).astype(np.float32)
        sinT = np.sin(ang).astype(np.float32)
        wqkv_np = np.ascontiguousarray(np.concatenate(
            [Wq[:, g * 1024:(g + 1) * 1024],
             Wk[:, g * 256:(g + 1) * 256],
             Wv[:, g * 256:(g + 1) * 256]], axis=1)).astype(ml_dtypes.bfloat16)
        wo_np = np.ascontiguousarray(
            Wo[g * 1024:(g + 1) * 1024, :]).astype(ml_dtypes.bfloat16)
        in_maps.append({"xT": xTb, "cosT": cosT, "sinT": sinT,
                        "wqkv": wqkv_np, "wo": wo_np})

    nc = _get_nc()
    res = run_bass_kernel_spmd(nc, in_maps, core_ids=list(range(8)),
                               trace=_TRACE)
    global _LAST_RES
    _LAST_RES = res
    outs = [r["out"].astype(np.float32) for r in res.results]
    return np.stack([outs[0] + outs[1] + outs[2] + outs[3],
                     outs[4] + outs[5] + outs[6] + outs[7]])
